# revision 72
# baseline (speedup 1.0000x reference)
"""Trainium2 Bass kernel for nn_MultiHeadAttention_83863531421896.

Full-input contract: kernel(**inputs) takes the unsharded tensors and
returns the full (2, 2048, 1024) output. Internally the 16 heads are
sharded 2-per-core across 8 NeuronCores (tensor parallel); each core
computes its heads' attention plus its slice of the output projection,
and the 8 partial projections are reduced on the host.

Device dataflow per core (heads h0, h1), all matmul operands bf16:
  per batch b:
    qkvT = W_qkv_slice @ x^T (PSUM accumulators live in the sc region,
           bias-adds split DVE/Scalar), V^T -> V via PE transposes,
           V packed as [V | ones] blocks (ones memset)
    attention per (q-chunk, kk-pair): both heads' scores as a
           64-row-quadrant PE pair into one 2-bank PSUM tile (triple
           buffered); per key tile the h0 half exps exactly on Scalar
           while the h1 half uses a one-op Schraudolph int16-bitcast-bf16
           approx exp on DVE (concurrent engines; separate es tiles
           avoid a write-after-write edge); A^T V as one 128-key-deep
           matmul per head into a single-bank accumulator + ones column
           for the softmax denominators; deferred normalization with
           approx reciprocal, normalize muls on GpSimd
    out-projection per m-chunk after each attention phase; batch-1 x
           prefetches during batch-0 attention
"""

import sys

if "/opt/trn_rl_repo" not in sys.path:
    sys.path.insert(0, "/opt/trn_rl_repo")

import numpy as np

B = 2
S = 2048
D = 1024
H = 16
HD = 64
N_CORES = 8
HEADS_PER_CORE = H // N_CORES  # 2
M = B * S                      # 4096 tokens
N_MCHUNK_B = S // 512          # 4 m-chunks of 512 tokens per batch
N_KTILE = D // 128             # 8 contraction tiles for qkv
N_QCHUNK = S // 512            # 4 q-chunks per batch
N_KKTILE = S // 128            # 16 key tiles per batch
SCALE = 1.0 / np.sqrt(HD)
# A couple of key tiles where Scalar exps both halves, to balance DVE's
# fixed eviction work
SCALAR_BOTH_KTS = (7,)
SCHR_A = SCALE * (2.0 ** 7) / np.log(2.0)
SCHR_B = 127.0 * 2 ** 7 - 0.043 * 2 ** 7

_CACHE = {}


def _build_module():
    import concourse.bass as bass
    import concourse.tile as tile
    from concourse import bacc, mybir

    f32 = mybir.dt.float32
    bf16 = mybir.dt.bfloat16
    i16 = mybir.dt.int16
    Exp = mybir.ActivationFunctionType.Exp
    Ident = mybir.ActivationFunctionType.Identity
    mult = mybir.AluOpType.mult
    add = mybir.AluOpType.add

    nc = bacc.Bacc("TRN2", target_bir_lowering=False, debug=False,
                   num_devices=N_CORES)

    xt_ap = nc.dram_tensor("xt", [D, M], bf16, kind="ExternalInput").ap()
    w_all_ap = nc.dram_tensor("w_all", [3 * D, 128], bf16,
                              kind="ExternalInput").ap()
    wo_ap = nc.dram_tensor("wo", [128, D], bf16, kind="ExternalInput").ap()
    b_all_ap = nc.dram_tensor("b_all", [128, 3], f32,
                              kind="ExternalInput").ap()
    ident_ap = nc.dram_tensor("ident", [128, 128], bf16,
                              kind="ExternalInput").ap()
    out_ap = nc.dram_tensor("partial", [D, M], bf16, kind="ExternalOutput").ap()
    sums_dram = nc.dram_tensor(
        "sums_scratch", [B * N_QCHUNK * HEADS_PER_CORE, 512], f32).ap()

    with tile.TileContext(nc) as tc:
        with tc.tile_pool(name="persist", bufs=1) as persist, \
             tc.tile_pool(name="const", bufs=1) as const, \
             tc.tile_pool(name="xpool", bufs=2) as xpool, \
             tc.tile_pool(name="vt_pool", bufs=2) as vt_pool, \
             tc.tile_pool(name="ps8", bufs=1, space="PSUM") as ps8, \
             tc.tile_pool(name="epool", bufs=2) as epool, \
             tc.tile_pool(name="stage", bufs=2) as stage, \
             tc.tile_pool(name="fin", bufs=4) as fin:
            qka_sb = persist.tile([128, M], bf16, tag="qka")
            qkb_sb = persist.tile([128, M], bf16, tag="qkb")
            v_sb = persist.tile([128, B, N_KKTILE, HEADS_PER_CORE, 65], bf16,
                                tag="vsb")
            outt_sb = persist.tile([128, M], bf16, tag="outt")

            nc.vector.memset(v_sb[:, :, :, :, 64:65], 1.0)
            # q-projection weights first so the first matmuls unblock early
            wq_sb = const.tile([128, 3, N_KTILE, 128], bf16, tag="wq")
            wv = w_all_ap.rearrange("(e ki p) c -> p e ki c", e=3, ki=N_KTILE)
            nc.gpsimd.dma_start(wq_sb[:, 0], wv[:, 0])
            b_sb = const.tile([128, 3], f32, tag="ball")
            nc.gpsimd.dma_start(b_sb[:], b_all_ap[:])
            nc.gpsimd.dma_start(wq_sb[:, 1], wv[:, 1])
            nc.gpsimd.dma_start(wq_sb[:, 2], wv[:, 2])
            ba_sb, bb_sb, bv_sb = (b_sb[:, e:e + 1] for e in range(3))
            wo_sb = const.tile([128, D], bf16, tag="wo")
            ident_sb = const.tile([128, 128], bf16, tag="ident")

            def sc_tile(name):
                return ps8.tile([128, 2, 512], f32, tag="sc", bufs=3,
                                name=name)

            def load_x(b2, eng):
                # one wide [128, 2048] DMA per ki (4KB contiguous rows)
                xs = xpool.tile([128, N_KTILE, S], bf16, tag="xs",
                                name=f"xs{b2}")
                for ki in range(N_KTILE):
                    e = eng if isinstance(eng, tuple) is False else \
                        eng[ki % len(eng)]
                    e.dma_start(xs[:, ki],
                                xt_ap[ki * 128:(ki + 1) * 128,
                                      b2 * S:(b2 + 1) * S])
                return xs

            def qkv_phase(b2, xs):
                vt_sb = vt_pool.tile([128, S], bf16, tag="vt", name=f"vt{b2}")
                # ki-inner-most over m-chunks: one weight load feeds 4
                # matmuls; accumulators are halves of two sc-region tiles
                for ei, (bias, dest) in enumerate(
                        [(ba_sb, qka_sb), (bb_sb, qkb_sb), (bv_sb, vt_sb)]):
                    psa = sc_tile(f"qkvpsa{ei}")
                    psb = sc_tile(f"qkvpsb{ei}")
                    pss = [psa[:, 0, :], psa[:, 1, :],
                           psb[:, 0, :], psb[:, 1, :]]
                    for ki in range(N_KTILE):
                        for mc in range(N_MCHUNK_B):
                            nc.tensor.matmul(pss[mc], wq_sb[:, ei, ki],
                                             xs[:, ki, mc * 512:(mc + 1) * 512],
                                             start=(ki == 0), stop=(ki == N_KTILE - 1))
                    for mc in range(N_MCHUNK_B):
                        col = (b2 * N_MCHUNK_B + mc) if ei < 2 else mc
                        dst = dest[:, col * 512:(col + 1) * 512]
                        if mc % 2 == 0:
                            nc.vector.tensor_scalar_add(dst, pss[mc], bias)
                        else:
                            nc.scalar.activation(dst, pss[mc], Ident,
                                                 bias=bias)
                for kt in range(N_KKTILE):
                    tp = ps8.tile([128, 128], bf16, tag=f"av{kt % 2}",
                                  name="tp")
                    nc.tensor.transpose(tp[:], vt_sb[:, kt * 128:(kt + 1) * 128],
                                        ident_sb[:])
                    for h in range(HEADS_PER_CORE):
                        nc.vector.tensor_copy(v_sb[:, b2, kt, h, 0:64],
                                              tp[:, h * 64:(h + 1) * 64])

            def outproj_chunk(b2, mc):
                # finalT[e, m-chunk] = wo_sb[:, e-tile].T @ outT[:, m-chunk]
                # 2-bank-wide PSUM; evacuations split DVE/Scalar
                mrow = b2 * S + mc * 512
                fo = fin.tile([128, D // 128, 512], bf16, tag="fo", bufs=2,
                              name="fo")
                for ep in range(D // 256):
                    fp = sc_tile("fp")
                    for sub in range(2):
                        et = ep * 2 + sub
                        nc.tensor.matmul(fp[:, sub, :],
                                         wo_sb[:, et * 128:(et + 1) * 128],
                                         outt_sb[:, mrow:mrow + 512],
                                         start=True, stop=True)
                    if ep % 2 == 0:
                        nc.vector.tensor_copy(fo[:, 2 * ep:2 * ep + 2, :],
                                              fp[:])
                    else:
                        nc.scalar.activation(fo[:, 2 * ep:2 * ep + 2, :],
                                             fp[:], Ident)
                eng = (nc.sync, nc.gpsimd, nc.scalar, nc.sync)[mc]
                eng.dma_start(
                    out_ap[:, mrow:mrow + 512].rearrange(
                        "(et p) c -> p et c", et=D // 128),
                    fo[:])

            def attn_phase(b2, prefetch=None):
                xss_next = None
                for qi in range(N_QCHUNK):
                    if qi == 0 and prefetch is not None:
                        xss_next = load_x(prefetch, (nc.sync, nc.gpsimd))
                    qcol = b2 * S + qi * 512
                    avp = [ps8.tile([128, 512], f32, tag=f"av{h}",
                                    name=f"av{h}")
                           for h in range(HEADS_PER_CORE)]

                    def emit_av(kt, es_kt):
                        first = (kt == 0)
                        last = (kt == N_KKTILE - 1)
                        for h in range(HEADS_PER_CORE):
                            nc.tensor.matmul(
                                avp[h][0:65, :],
                                v_sb[:, b2, kt, h, :],
                                es_kt[h][:, :],
                                start=first, stop=last)

                    pending = []
                    for kt in range(N_KKTILE):
                        kkcol = b2 * S + kt * 128
                        sc = sc_tile("sc")
                        for h in range(HEADS_PER_CORE):
                            nc.tensor.matmul(
                                sc[:, h, :],
                                qkb_sb[h * 64:(h + 1) * 64, kkcol:kkcol + 128],
                                qka_sb[h * 64:(h + 1) * 64, qcol:qcol + 512],
                                start=True, stop=True)
                        # separate tiles per head-half: a shared tile would
                        # put a write-after-write edge between the Scalar exp
                        # and the DVE approx exp and serialize the engines
                        e0 = epool.tile([128, 512], bf16, tag="e0",
                                        bufs=7, name="e0")
                        e1 = epool.tile([128, 512], bf16, tag="e1",
                                        bufs=7, name="e1")
                        nc.scalar.activation(e0[:], sc[:, 0, :], Exp,
                                             scale=SCALE)
                        if kt in SCALAR_BOTH_KTS:
                            nc.scalar.activation(e1[:], sc[:, 1, :], Exp,
                                                 scale=SCALE)
                        else:
                            nc.vector.tensor_scalar(
                                e1[:].bitcast(i16), sc[:, 1, :],
                                SCHR_A, SCHR_B, mult, add)
                        pending.append((kt, (e0, e1)))
                        if len(pending) > 4:
                            emit_av(*pending.pop(0))
                    while pending:
                        emit_av(*pending.pop(0))

                    sidx = (b2 * N_QCHUNK + qi) * HEADS_PER_CORE
                    sts = []
                    for h in range(HEADS_PER_CORE):
                        st = stage.tile([128, 512], f32, tag=f"st{h}",
                                        name="st")
                        nc.scalar.activation(st[0:65, :], avp[h][0:65, :],
                                             Ident)
                        nc.gpsimd.dma_start(sums_dram[sidx + h:sidx + h + 1, :],
                                            st[64:65, :])
                        sts.append(st)
                    for h in range(HEADS_PER_CORE):
                        st = sts[h]
                        rb = stage.tile([128, 512], f32, tag="rb", name="rb")
                        nc.gpsimd.dma_start(
                            rb[0:64, :],
                            sums_dram[sidx + h:sidx + h + 1, :]
                            .partition_broadcast(64).squeeze(1))
                        rb2 = stage.tile([128, 512], f32, tag="rb2", name="rb2")
                        nc.vector.reciprocal_approx_fast(rb2[0:64, :],
                                                         rb[0:64, :])
                        # all-SBUF operands: the normalize muls can run on
                        # the otherwise-idle GpSimd engine
                        if h == 0:
                            nc.gpsimd.tensor_mul(
                                outt_sb[0:64, qcol:qcol + 512],
                                st[0:64, :], rb2[0:64, :])
                        else:
                            tm = stage.tile([128, 512], bf16, tag="tm",
                                            name="tm")
                            nc.gpsimd.tensor_mul(tm[0:64, :], st[0:64, :],
                                                 rb2[0:64, :])
                            nc.gpsimd.dma_start(
                                outt_sb[64:128, qcol:qcol + 512], tm[0:64, :])
                return xss_next

            xss0 = load_x(0, (nc.sync, nc.scalar))
            nc.scalar.dma_start(ident_sb[:], ident_ap[:])
            nc.scalar.dma_start(wo_sb[:], wo_ap[:])
            qkv_phase(0, xss0)
            xss1 = attn_phase(0, prefetch=1)
            qkv_phase(1, xss1)
            for mc in range(N_MCHUNK_B):
                outproj_chunk(0, mc)
            tc.no_sync_barrier()
            attn_phase(1)
            tc.no_sync_barrier()
            for mc in range(N_MCHUNK_B):
                outproj_chunk(1, mc)
    nc.compile()
    return nc


def _shard_inputs(x, w_qkv, b_qkv, w_out):
    import ml_dtypes

    bf = ml_dtypes.bfloat16
    xt = np.ascontiguousarray(x.reshape(M, D).T).astype(bf)  # (1024, 4096)
    ident = np.eye(128, dtype=bf)
    in_maps = []
    for c in range(N_CORES):
        h0 = HEADS_PER_CORE * c
        rows_q, rows_k, rows_v, dcols = [], [], [], []
        for h in (h0, h0 + 1):
            rows_q += list(range(h * 192, h * 192 + 64))
            rows_k += list(range(h * 192 + 64, h * 192 + 128))
            rows_v += list(range(h * 192 + 128, h * 192 + 192))
            dcols += list(range(h * 64, (h + 1) * 64))
        w_all = np.concatenate([
            np.ascontiguousarray(w_qkv[rows_q, :].T),
            np.ascontiguousarray(w_qkv[rows_k, :].T),
            np.ascontiguousarray(w_qkv[rows_v, :].T)], axis=0).astype(bf)
        b_all = np.stack([b_qkv[rows_q], b_qkv[rows_k], b_qkv[rows_v]],
                         axis=1).astype(np.float32)
        in_maps.append({
            "xt": xt,
            "w_all": np.ascontiguousarray(w_all),
            "wo": np.ascontiguousarray(w_out[:, dcols].T).astype(bf),
            "b_all": np.ascontiguousarray(b_all),
            "ident": ident,
        })
    return in_maps


def kernel(x, w_qkv, b_qkv, w_out, b_out, _trace=False):
    from concourse.bass_utils import run_bass_kernel_spmd

    x = np.asarray(x, dtype=np.float32)
    w_qkv = np.asarray(w_qkv, dtype=np.float32)
    b_qkv = np.asarray(b_qkv, dtype=np.float32)
    w_out = np.asarray(w_out, dtype=np.float32)
    b_out = np.asarray(b_out, dtype=np.float32)

    if "nc" not in _CACHE:
        _CACHE["nc"] = _build_module()
    nc = _CACHE["nc"]

    in_maps = _shard_inputs(x, w_qkv, b_qkv, w_out)
    res = run_bass_kernel_spmd(nc, in_maps, list(range(N_CORES)), trace=_trace)
    acc = np.zeros((D, M), dtype=np.float64)
    for c in range(N_CORES):
        acc += res.results[c]["partial"].astype(np.float32)
    acc = acc.T + b_out
    out = acc.astype(np.float32).reshape(B, S, D)
    if _trace:
        _CACHE["last_exec_time_ns"] = res.exec_time_ns
        _CACHE["last_res"] = res
    return out


# revision 73
# speedup vs baseline: 1.0218x; 1.0218x over previous
"""Trainium2 Bass kernel for nn_MultiHeadAttention_83863531421896.

Full-input contract: kernel(**inputs) takes the unsharded tensors and
returns the full (2, 2048, 1024) output. Internally the 16 heads are
sharded 2-per-core across 8 NeuronCores (tensor parallel); each core
computes its heads' attention plus its slice of the output projection,
and the 8 partial projections are reduced on the host.

Device dataflow per core (heads h0, h1), all matmul operands bf16:
  per batch b:
    qkvT = W_qkv_slice @ x^T (PSUM accumulators live in the sc region,
           bias-adds split DVE/Scalar), V^T -> V via PE transposes,
           V packed as [V | ones] blocks (ones memset)
    attention per (q-chunk, kk-pair): both heads' scores as a
           64-row-quadrant PE pair into one 2-bank PSUM tile (triple
           buffered); per key tile the h0 half exps exactly on Scalar
           while the h1 half uses a one-op Schraudolph int16-bitcast-bf16
           approx exp on DVE (concurrent engines; separate es tiles
           avoid a write-after-write edge); A^T V as one 128-key-deep
           matmul per head into a single-bank accumulator + ones column
           for the softmax denominators; deferred normalization with
           approx reciprocal, normalize muls on GpSimd
    out-projection per m-chunk after each attention phase; batch-1 x
           prefetches during batch-0 attention
"""

import sys

if "/opt/trn_rl_repo" not in sys.path:
    sys.path.insert(0, "/opt/trn_rl_repo")

import numpy as np

B = 2
S = 2048
D = 1024
H = 16
HD = 64
N_CORES = 8
HEADS_PER_CORE = H // N_CORES  # 2
M = B * S                      # 4096 tokens
N_MCHUNK_B = S // 512          # 4 m-chunks of 512 tokens per batch
N_KTILE = D // 128             # 8 contraction tiles for qkv
N_QCHUNK = S // 512            # 4 q-chunks per batch
N_KKTILE = S // 128            # 16 key tiles per batch
SCALE = 1.0 / np.sqrt(HD)
# A couple of key tiles where Scalar exps both halves, to balance DVE's
# fixed eviction work
SCALAR_BOTH_KTS = (5, 10)
SCHR_A = SCALE * (2.0 ** 7) / np.log(2.0)
SCHR_B = 127.0 * 2 ** 7 - 0.043 * 2 ** 7

_CACHE = {}


def _build_module():
    import concourse.bass as bass
    import concourse.tile as tile
    from concourse import bacc, mybir

    f32 = mybir.dt.float32
    bf16 = mybir.dt.bfloat16
    i16 = mybir.dt.int16
    Exp = mybir.ActivationFunctionType.Exp
    Ident = mybir.ActivationFunctionType.Identity
    mult = mybir.AluOpType.mult
    add = mybir.AluOpType.add

    nc = bacc.Bacc("TRN2", target_bir_lowering=False, debug=False,
                   num_devices=N_CORES)

    xt_ap = nc.dram_tensor("xt", [D, M], bf16, kind="ExternalInput").ap()
    w_all_ap = nc.dram_tensor("w_all", [3 * D, 128], bf16,
                              kind="ExternalInput").ap()
    wo_ap = nc.dram_tensor("wo", [128, D], bf16, kind="ExternalInput").ap()
    b_all_ap = nc.dram_tensor("b_all", [128, 3], f32,
                              kind="ExternalInput").ap()
    ident_ap = nc.dram_tensor("ident", [128, 128], bf16,
                              kind="ExternalInput").ap()
    out_ap = nc.dram_tensor("partial", [D, M], bf16, kind="ExternalOutput").ap()
    sums_dram = nc.dram_tensor(
        "sums_scratch", [B * N_QCHUNK * HEADS_PER_CORE, 512], f32).ap()

    with tile.TileContext(nc) as tc:
        with tc.tile_pool(name="persist", bufs=1) as persist, \
             tc.tile_pool(name="const", bufs=1) as const, \
             tc.tile_pool(name="xpool", bufs=2) as xpool, \
             tc.tile_pool(name="vt_pool", bufs=2) as vt_pool, \
             tc.tile_pool(name="ps8", bufs=1, space="PSUM") as ps8, \
             tc.tile_pool(name="epool", bufs=2) as epool, \
             tc.tile_pool(name="stage", bufs=2) as stage, \
             tc.tile_pool(name="fin", bufs=4) as fin:
            qka_sb = persist.tile([128, M], bf16, tag="qka")
            qkb_sb = persist.tile([128, M], bf16, tag="qkb")
            v_sb = persist.tile([128, B, N_KKTILE, HEADS_PER_CORE, 65], bf16,
                                tag="vsb")
            outt_sb = persist.tile([128, M], bf16, tag="outt")

            nc.vector.memset(v_sb[:, :, :, :, 64:65], 1.0)
            # q-projection weights first so the first matmuls unblock early
            wq_sb = const.tile([128, 3, N_KTILE, 128], bf16, tag="wq")
            wv = w_all_ap.rearrange("(e ki p) c -> p e ki c", e=3, ki=N_KTILE)
            nc.gpsimd.dma_start(wq_sb[:, 0], wv[:, 0])
            b_sb = const.tile([128, 3], f32, tag="ball")
            nc.gpsimd.dma_start(b_sb[:], b_all_ap[:])
            nc.gpsimd.dma_start(wq_sb[:, 1], wv[:, 1])
            nc.gpsimd.dma_start(wq_sb[:, 2], wv[:, 2])
            ba_sb, bb_sb, bv_sb = (b_sb[:, e:e + 1] for e in range(3))
            wo_sb = const.tile([128, D], bf16, tag="wo")
            ident_sb = const.tile([128, 128], bf16, tag="ident")

            def sc_tile(name):
                return ps8.tile([128, 2, 512], f32, tag="sc", bufs=3,
                                name=name)

            def load_x(b2, eng):
                # one wide [128, 2048] DMA per ki (4KB contiguous rows)
                xs = xpool.tile([128, N_KTILE, S], bf16, tag="xs",
                                name=f"xs{b2}")
                for ki in range(N_KTILE):
                    e = eng if isinstance(eng, tuple) is False else \
                        eng[ki % len(eng)]
                    e.dma_start(xs[:, ki],
                                xt_ap[ki * 128:(ki + 1) * 128,
                                      b2 * S:(b2 + 1) * S])
                return xs

            def qkv_phase(b2, xs):
                vt_sb = vt_pool.tile([128, S], bf16, tag="vt", name=f"vt{b2}")
                # ki-inner-most over m-chunks: one weight load feeds 4
                # matmuls; accumulators are halves of two sc-region tiles
                for ei, (bias, dest) in enumerate(
                        [(ba_sb, qka_sb), (bb_sb, qkb_sb), (bv_sb, vt_sb)]):
                    psa = sc_tile(f"qkvpsa{ei}")
                    psb = sc_tile(f"qkvpsb{ei}")
                    pss = [psa[:, 0, :], psa[:, 1, :],
                           psb[:, 0, :], psb[:, 1, :]]
                    for ki in range(N_KTILE):
                        for mc in range(N_MCHUNK_B):
                            nc.tensor.matmul(pss[mc], wq_sb[:, ei, ki],
                                             xs[:, ki, mc * 512:(mc + 1) * 512],
                                             start=(ki == 0), stop=(ki == N_KTILE - 1))
                    for mc in range(N_MCHUNK_B):
                        col = (b2 * N_MCHUNK_B + mc) if ei < 2 else mc
                        dst = dest[:, col * 512:(col + 1) * 512]
                        if mc % 2 == 0:
                            nc.vector.tensor_scalar_add(dst, pss[mc], bias)
                        else:
                            nc.scalar.activation(dst, pss[mc], Ident,
                                                 bias=bias)
                for kt in range(N_KKTILE):
                    tp = ps8.tile([128, 128], bf16, tag=f"av{kt % 2}",
                                  name="tp")
                    nc.tensor.transpose(tp[:], vt_sb[:, kt * 128:(kt + 1) * 128],
                                        ident_sb[:])
                    for h in range(HEADS_PER_CORE):
                        nc.vector.tensor_copy(v_sb[:, b2, kt, h, 0:64],
                                              tp[:, h * 64:(h + 1) * 64])

            def outproj_chunk(b2, mc):
                # finalT[e, m-chunk] = wo_sb[:, e-tile].T @ outT[:, m-chunk]
                # 2-bank-wide PSUM; evacuations split DVE/Scalar
                mrow = b2 * S + mc * 512
                fo = fin.tile([128, D // 128, 512], bf16, tag="fo", bufs=2,
                              name="fo")
                for ep in range(D // 256):
                    fp = sc_tile("fp")
                    for sub in range(2):
                        et = ep * 2 + sub
                        nc.tensor.matmul(fp[:, sub, :],
                                         wo_sb[:, et * 128:(et + 1) * 128],
                                         outt_sb[:, mrow:mrow + 512],
                                         start=True, stop=True)
                    if ep % 2 == 0:
                        nc.vector.tensor_copy(fo[:, 2 * ep:2 * ep + 2, :],
                                              fp[:])
                    else:
                        nc.scalar.activation(fo[:, 2 * ep:2 * ep + 2, :],
                                             fp[:], Ident)
                eng = (nc.sync, nc.gpsimd, nc.scalar, nc.sync)[mc]
                eng.dma_start(
                    out_ap[:, mrow:mrow + 512].rearrange(
                        "(et p) c -> p et c", et=D // 128),
                    fo[:])

            def attn_phase(b2, prefetch=None):
                xss_next = None
                for qi in range(N_QCHUNK):
                    if qi == 0 and prefetch is not None:
                        xss_next = load_x(prefetch, (nc.sync, nc.gpsimd))
                    qcol = b2 * S + qi * 512
                    avp = [ps8.tile([128, 512], f32, tag=f"av{h}",
                                    name=f"av{h}")
                           for h in range(HEADS_PER_CORE)]

                    def emit_av(kt, es_kt):
                        first = (kt == 0)
                        last = (kt == N_KKTILE - 1)
                        for h in range(HEADS_PER_CORE):
                            nc.tensor.matmul(
                                avp[h][0:65, :],
                                v_sb[:, b2, kt, h, :],
                                es_kt[h][:, :],
                                start=first, stop=last)

                    pending = []
                    for kt in range(N_KKTILE):
                        kkcol = b2 * S + kt * 128
                        sc = sc_tile("sc")
                        for h in range(HEADS_PER_CORE):
                            nc.tensor.matmul(
                                sc[:, h, :],
                                qkb_sb[h * 64:(h + 1) * 64, kkcol:kkcol + 128],
                                qka_sb[h * 64:(h + 1) * 64, qcol:qcol + 512],
                                start=True, stop=True)
                        # separate tiles per head-half: a shared tile would
                        # put a write-after-write edge between the Scalar exp
                        # and the DVE approx exp and serialize the engines
                        e0 = epool.tile([128, 512], bf16, tag="e0",
                                        bufs=7, name="e0")
                        e1 = epool.tile([128, 512], bf16, tag="e1",
                                        bufs=7, name="e1")
                        nc.scalar.activation(e0[:], sc[:, 0, :], Exp,
                                             scale=SCALE)
                        if kt in SCALAR_BOTH_KTS:
                            nc.scalar.activation(e1[:], sc[:, 1, :], Exp,
                                                 scale=SCALE)
                        else:
                            nc.vector.tensor_scalar(
                                e1[:].bitcast(i16), sc[:, 1, :],
                                SCHR_A, SCHR_B, mult, add)
                        pending.append((kt, (e0, e1)))
                        if len(pending) > 4:
                            emit_av(*pending.pop(0))
                    while pending:
                        emit_av(*pending.pop(0))

                    sidx = (b2 * N_QCHUNK + qi) * HEADS_PER_CORE
                    sts = []
                    for h in range(HEADS_PER_CORE):
                        st = stage.tile([128, 512], f32, tag=f"st{h}",
                                        name="st")
                        if h == 0:
                            nc.scalar.activation(st[0:65, :], avp[h][0:65, :],
                                                 Ident)
                        else:
                            nc.vector.tensor_copy(st[0:65, :], avp[h][0:65, :])
                        nc.gpsimd.dma_start(sums_dram[sidx + h:sidx + h + 1, :],
                                            st[64:65, :])
                        sts.append(st)
                    for h in range(HEADS_PER_CORE):
                        st = sts[h]
                        rb = stage.tile([128, 512], f32, tag="rb", name="rb")
                        nc.gpsimd.dma_start(
                            rb[0:64, :],
                            sums_dram[sidx + h:sidx + h + 1, :]
                            .partition_broadcast(64).squeeze(1))
                        rb2 = stage.tile([128, 512], f32, tag="rb2", name="rb2")
                        nc.vector.reciprocal_approx_fast(rb2[0:64, :],
                                                         rb[0:64, :])
                        # all-SBUF operands: the normalize muls can run on
                        # the otherwise-idle GpSimd engine
                        if h == 0:
                            nc.gpsimd.tensor_mul(
                                outt_sb[0:64, qcol:qcol + 512],
                                st[0:64, :], rb2[0:64, :])
                        else:
                            tm = stage.tile([128, 512], bf16, tag="tm",
                                            name="tm")
                            nc.gpsimd.tensor_mul(tm[0:64, :], st[0:64, :],
                                                 rb2[0:64, :])
                            nc.gpsimd.dma_start(
                                outt_sb[64:128, qcol:qcol + 512], tm[0:64, :])
                return xss_next

            xss0 = load_x(0, (nc.sync, nc.scalar))
            nc.scalar.dma_start(ident_sb[:], ident_ap[:])
            nc.scalar.dma_start(wo_sb[:], wo_ap[:])
            qkv_phase(0, xss0)
            xss1 = attn_phase(0, prefetch=1)
            qkv_phase(1, xss1)
            for mc in range(N_MCHUNK_B):
                outproj_chunk(0, mc)
            tc.no_sync_barrier()
            attn_phase(1)
            tc.no_sync_barrier()
            for mc in range(N_MCHUNK_B):
                outproj_chunk(1, mc)
    nc.compile()
    return nc


def _shard_inputs(x, w_qkv, b_qkv, w_out):
    import ml_dtypes

    bf = ml_dtypes.bfloat16
    xt = np.ascontiguousarray(x.reshape(M, D).T).astype(bf)  # (1024, 4096)
    ident = np.eye(128, dtype=bf)
    in_maps = []
    for c in range(N_CORES):
        h0 = HEADS_PER_CORE * c
        rows_q, rows_k, rows_v, dcols = [], [], [], []
        for h in (h0, h0 + 1):
            rows_q += list(range(h * 192, h * 192 + 64))
            rows_k += list(range(h * 192 + 64, h * 192 + 128))
            rows_v += list(range(h * 192 + 128, h * 192 + 192))
            dcols += list(range(h * 64, (h + 1) * 64))
        w_all = np.concatenate([
            np.ascontiguousarray(w_qkv[rows_q, :].T),
            np.ascontiguousarray(w_qkv[rows_k, :].T),
            np.ascontiguousarray(w_qkv[rows_v, :].T)], axis=0).astype(bf)
        b_all = np.stack([b_qkv[rows_q], b_qkv[rows_k], b_qkv[rows_v]],
                         axis=1).astype(np.float32)
        in_maps.append({
            "xt": xt,
            "w_all": np.ascontiguousarray(w_all),
            "wo": np.ascontiguousarray(w_out[:, dcols].T).astype(bf),
            "b_all": np.ascontiguousarray(b_all),
            "ident": ident,
        })
    return in_maps


def kernel(x, w_qkv, b_qkv, w_out, b_out, _trace=False):
    from concourse.bass_utils import run_bass_kernel_spmd

    x = np.asarray(x, dtype=np.float32)
    w_qkv = np.asarray(w_qkv, dtype=np.float32)
    b_qkv = np.asarray(b_qkv, dtype=np.float32)
    w_out = np.asarray(w_out, dtype=np.float32)
    b_out = np.asarray(b_out, dtype=np.float32)

    if "nc" not in _CACHE:
        _CACHE["nc"] = _build_module()
    nc = _CACHE["nc"]

    in_maps = _shard_inputs(x, w_qkv, b_qkv, w_out)
    res = run_bass_kernel_spmd(nc, in_maps, list(range(N_CORES)), trace=_trace)
    acc = np.zeros((D, M), dtype=np.float64)
    for c in range(N_CORES):
        acc += res.results[c]["partial"].astype(np.float32)
    acc = acc.T + b_out
    out = acc.astype(np.float32).reshape(B, S, D)
    if _trace:
        _CACHE["last_exec_time_ns"] = res.exec_time_ns
        _CACHE["last_res"] = res
    return out


# revision 74
# speedup vs baseline: 1.0353x; 1.0131x over previous
"""Trainium2 Bass kernel for nn_MultiHeadAttention_83863531421896.

Full-input contract: kernel(**inputs) takes the unsharded tensors and
returns the full (2, 2048, 1024) output. Internally the 16 heads are
sharded 2-per-core across 8 NeuronCores (tensor parallel); each core
computes its heads' attention plus its slice of the output projection,
and the 8 partial projections are reduced on the host.

Device dataflow per core (heads h0, h1), all matmul operands bf16:
  per batch b:
    qkvT = W_qkv_slice @ x^T (PSUM accumulators live in the sc region,
           bias-adds split DVE/Scalar), V^T -> V via PE transposes,
           V packed as [V | ones] blocks (ones memset)
    attention per (q-chunk, kk-pair): both heads' scores as a
           64-row-quadrant PE pair into one 2-bank PSUM tile (triple
           buffered); per key tile the h0 half exps exactly on Scalar
           while the h1 half uses a one-op Schraudolph int16-bitcast-bf16
           approx exp on DVE (concurrent engines; separate es tiles
           avoid a write-after-write edge); A^T V as one 128-key-deep
           matmul per head into a single-bank accumulator + ones column
           for the softmax denominators; deferred normalization with
           approx reciprocal, normalize muls on GpSimd
    out-projection per m-chunk after each attention phase; batch-1 x
           prefetches during batch-0 attention
"""

import sys

if "/opt/trn_rl_repo" not in sys.path:
    sys.path.insert(0, "/opt/trn_rl_repo")

import numpy as np

B = 2
S = 2048
D = 1024
H = 16
HD = 64
N_CORES = 8
HEADS_PER_CORE = H // N_CORES  # 2
M = B * S                      # 4096 tokens
N_MCHUNK_B = S // 512          # 4 m-chunks of 512 tokens per batch
N_KTILE = D // 128             # 8 contraction tiles for qkv
N_QCHUNK = S // 512            # 4 q-chunks per batch
N_KKTILE = S // 128            # 16 key tiles per batch
SCALE = 1.0 / np.sqrt(HD)
# A couple of key tiles where Scalar exps both halves, to balance DVE's
# fixed eviction work
SCALAR_BOTH_KTS = (5, 10)
SCHR_A = SCALE * (2.0 ** 7) / np.log(2.0)
SCHR_B = 127.0 * 2 ** 7 - 0.043 * 2 ** 7

_CACHE = {}


def _build_module():
    import concourse.bass as bass
    import concourse.tile as tile
    from concourse import bacc, mybir

    f32 = mybir.dt.float32
    bf16 = mybir.dt.bfloat16
    i16 = mybir.dt.int16
    Exp = mybir.ActivationFunctionType.Exp
    Ident = mybir.ActivationFunctionType.Identity
    mult = mybir.AluOpType.mult
    add = mybir.AluOpType.add

    nc = bacc.Bacc("TRN2", target_bir_lowering=False, debug=False,
                   num_devices=N_CORES)

    xt_ap = nc.dram_tensor("xt", [D, M], bf16, kind="ExternalInput").ap()
    w_all_ap = nc.dram_tensor("w_all", [3 * D, 128], bf16,
                              kind="ExternalInput").ap()
    wo_ap = nc.dram_tensor("wo", [128, D], bf16, kind="ExternalInput").ap()
    b_all_ap = nc.dram_tensor("b_all", [128, 3], f32,
                              kind="ExternalInput").ap()
    ident_ap = nc.dram_tensor("ident", [128, 128], bf16,
                              kind="ExternalInput").ap()
    out_ap = nc.dram_tensor("partial", [D, M], bf16, kind="ExternalOutput").ap()
    sums_dram = nc.dram_tensor(
        "sums_scratch", [B * N_QCHUNK * HEADS_PER_CORE, 512], f32).ap()

    with tile.TileContext(nc) as tc:
        with tc.tile_pool(name="persist", bufs=1) as persist, \
             tc.tile_pool(name="const", bufs=1) as const, \
             tc.tile_pool(name="xpool", bufs=2) as xpool, \
             tc.tile_pool(name="vt_pool", bufs=2) as vt_pool, \
             tc.tile_pool(name="ps8", bufs=1, space="PSUM") as ps8, \
             tc.tile_pool(name="epool", bufs=2) as epool, \
             tc.tile_pool(name="stage", bufs=2) as stage, \
             tc.tile_pool(name="fin", bufs=4) as fin:
            qka_sb = persist.tile([128, M], bf16, tag="qka")
            qkb_sb = persist.tile([128, M], bf16, tag="qkb")
            v_sb = persist.tile([128, B, N_KKTILE, HEADS_PER_CORE, 65], bf16,
                                tag="vsb")
            outt_sb = persist.tile([128, M], bf16, tag="outt")

            nc.vector.memset(v_sb[:, :, :, :, 64:65], 1.0)
            # q-projection weights first so the first matmuls unblock early
            wq_sb = const.tile([128, 3, N_KTILE, 128], bf16, tag="wq")
            wv = w_all_ap.rearrange("(e ki p) c -> p e ki c", e=3, ki=N_KTILE)
            nc.gpsimd.dma_start(wq_sb[:, 0], wv[:, 0])
            b_sb = const.tile([128, 3], f32, tag="ball")
            nc.gpsimd.dma_start(b_sb[:], b_all_ap[:])
            nc.gpsimd.dma_start(wq_sb[:, 1], wv[:, 1])
            nc.gpsimd.dma_start(wq_sb[:, 2], wv[:, 2])
            ba_sb, bb_sb, bv_sb = (b_sb[:, e:e + 1] for e in range(3))
            wo_sb = const.tile([128, D], bf16, tag="wo")
            ident_sb = const.tile([128, 128], bf16, tag="ident")

            def sc_tile(name):
                return ps8.tile([128, 2, 512], f32, tag="sc", bufs=3,
                                name=name)

            def load_x(b2, eng):
                # one wide [128, 2048] DMA per ki (4KB contiguous rows)
                xs = xpool.tile([128, N_KTILE, S], bf16, tag="xs",
                                name=f"xs{b2}")
                for ki in range(N_KTILE):
                    if isinstance(eng, tuple):
                        e = eng[ki % len(eng)] if len(eng) != 3 else \
                            (eng[0], eng[1], eng[0], eng[1], eng[0], eng[1],
                             eng[2], eng[2])[ki]
                    else:
                        e = eng
                    e.dma_start(xs[:, ki],
                                xt_ap[ki * 128:(ki + 1) * 128,
                                      b2 * S:(b2 + 1) * S])
                return xs

            def qkv_phase(b2, xs):
                vt_sb = vt_pool.tile([128, S], bf16, tag="vt", name=f"vt{b2}")
                # ki-inner-most over m-chunks: one weight load feeds 4
                # matmuls; accumulators are halves of two sc-region tiles
                for ei, (bias, dest) in enumerate(
                        [(ba_sb, qka_sb), (bb_sb, qkb_sb), (bv_sb, vt_sb)]):
                    psa = sc_tile(f"qkvpsa{ei}")
                    psb = sc_tile(f"qkvpsb{ei}")
                    pss = [psa[:, 0, :], psa[:, 1, :],
                           psb[:, 0, :], psb[:, 1, :]]
                    for ki in range(N_KTILE):
                        for mc in range(N_MCHUNK_B):
                            nc.tensor.matmul(pss[mc], wq_sb[:, ei, ki],
                                             xs[:, ki, mc * 512:(mc + 1) * 512],
                                             start=(ki == 0), stop=(ki == N_KTILE - 1))
                    for mc in range(N_MCHUNK_B):
                        col = (b2 * N_MCHUNK_B + mc) if ei < 2 else mc
                        dst = dest[:, col * 512:(col + 1) * 512]
                        if mc % 2 == 0:
                            nc.vector.tensor_scalar_add(dst, pss[mc], bias)
                        else:
                            nc.scalar.activation(dst, pss[mc], Ident,
                                                 bias=bias)
                for kt in range(N_KKTILE):
                    tp = ps8.tile([128, 128], bf16, tag=f"av{kt % 2}",
                                  name="tp")
                    nc.tensor.transpose(tp[:], vt_sb[:, kt * 128:(kt + 1) * 128],
                                        ident_sb[:])
                    for h in range(HEADS_PER_CORE):
                        nc.vector.tensor_copy(v_sb[:, b2, kt, h, 0:64],
                                              tp[:, h * 64:(h + 1) * 64])

            def outproj_chunk(b2, mc):
                # finalT[e, m-chunk] = wo_sb[:, e-tile].T @ outT[:, m-chunk]
                # 2-bank-wide PSUM; evacuations split DVE/Scalar
                mrow = b2 * S + mc * 512
                fo = fin.tile([128, D // 128, 512], bf16, tag="fo", bufs=2,
                              name="fo")
                for ep in range(D // 256):
                    fp = sc_tile("fp")
                    for sub in range(2):
                        et = ep * 2 + sub
                        nc.tensor.matmul(fp[:, sub, :],
                                         wo_sb[:, et * 128:(et + 1) * 128],
                                         outt_sb[:, mrow:mrow + 512],
                                         start=True, stop=True)
                    if ep % 2 == 0:
                        nc.vector.tensor_copy(fo[:, 2 * ep:2 * ep + 2, :],
                                              fp[:])
                    else:
                        nc.scalar.activation(fo[:, 2 * ep:2 * ep + 2, :],
                                             fp[:], Ident)
                # two half-chunk DMAs on different queues: a single 1MB
                # transfer serializes ~8.5us on one queue
                engs = ((nc.sync, nc.gpsimd), (nc.scalar, nc.sync),
                        (nc.gpsimd, nc.scalar), (nc.sync, nc.gpsimd))[mc]
                half = out_ap[:, mrow:mrow + 512].rearrange(
                    "(et p) c -> p et c", et=D // 128)
                engs[0].dma_start(half[:, 0:4], fo[:, 0:4])
                engs[1].dma_start(half[:, 4:8], fo[:, 4:8])

            def attn_phase(b2, prefetch=None):
                xss_next = None
                for qi in range(N_QCHUNK):
                    if qi == 0 and prefetch is not None:
                        xss_next = load_x(prefetch, (nc.sync, nc.gpsimd))
                    qcol = b2 * S + qi * 512
                    avp = [ps8.tile([128, 512], f32, tag=f"av{h}",
                                    name=f"av{h}")
                           for h in range(HEADS_PER_CORE)]

                    def emit_av(kt, es_kt):
                        first = (kt == 0)
                        last = (kt == N_KKTILE - 1)
                        for h in range(HEADS_PER_CORE):
                            nc.tensor.matmul(
                                avp[h][0:65, :],
                                v_sb[:, b2, kt, h, :],
                                es_kt[h][:, :],
                                start=first, stop=last)

                    pending = []
                    for kt in range(N_KKTILE):
                        kkcol = b2 * S + kt * 128
                        sc = sc_tile("sc")
                        for h in range(HEADS_PER_CORE):
                            nc.tensor.matmul(
                                sc[:, h, :],
                                qkb_sb[h * 64:(h + 1) * 64, kkcol:kkcol + 128],
                                qka_sb[h * 64:(h + 1) * 64, qcol:qcol + 512],
                                start=True, stop=True)
                        # separate tiles per head-half: a shared tile would
                        # put a write-after-write edge between the Scalar exp
                        # and the DVE approx exp and serialize the engines
                        e0 = epool.tile([128, 512], bf16, tag="e0",
                                        bufs=7, name="e0")
                        e1 = epool.tile([128, 512], bf16, tag="e1",
                                        bufs=7, name="e1")
                        nc.scalar.activation(e0[:], sc[:, 0, :], Exp,
                                             scale=SCALE)
                        if kt in SCALAR_BOTH_KTS:
                            nc.scalar.activation(e1[:], sc[:, 1, :], Exp,
                                                 scale=SCALE)
                        else:
                            nc.vector.tensor_scalar(
                                e1[:].bitcast(i16), sc[:, 1, :],
                                SCHR_A, SCHR_B, mult, add)
                        pending.append((kt, (e0, e1)))
                        if len(pending) > 4:
                            emit_av(*pending.pop(0))
                    while pending:
                        emit_av(*pending.pop(0))

                    sidx = (b2 * N_QCHUNK + qi) * HEADS_PER_CORE
                    sts = []
                    for h in range(HEADS_PER_CORE):
                        st = stage.tile([128, 512], f32, tag=f"st{h}",
                                        name="st")
                        if h == 0:
                            nc.scalar.activation(st[0:65, :], avp[h][0:65, :],
                                                 Ident)
                        else:
                            nc.vector.tensor_copy(st[0:65, :], avp[h][0:65, :])
                        nc.gpsimd.dma_start(sums_dram[sidx + h:sidx + h + 1, :],
                                            st[64:65, :])
                        sts.append(st)
                    for h in range(HEADS_PER_CORE):
                        st = sts[h]
                        rb = stage.tile([128, 512], f32, tag="rb", name="rb")
                        nc.gpsimd.dma_start(
                            rb[0:64, :],
                            sums_dram[sidx + h:sidx + h + 1, :]
                            .partition_broadcast(64).squeeze(1))
                        rb2 = stage.tile([128, 512], f32, tag="rb2", name="rb2")
                        nc.vector.reciprocal_approx_fast(rb2[0:64, :],
                                                         rb[0:64, :])
                        # all-SBUF operands: the normalize muls can run on
                        # the otherwise-idle GpSimd engine
                        if h == 0:
                            nc.gpsimd.tensor_mul(
                                outt_sb[0:64, qcol:qcol + 512],
                                st[0:64, :], rb2[0:64, :])
                        else:
                            tm = stage.tile([128, 512], bf16, tag="tm",
                                            name="tm")
                            nc.gpsimd.tensor_mul(tm[0:64, :], st[0:64, :],
                                                 rb2[0:64, :])
                            nc.gpsimd.dma_start(
                                outt_sb[64:128, qcol:qcol + 512], tm[0:64, :])
                return xss_next

            xss0 = load_x(0, (nc.sync, nc.scalar, nc.gpsimd))
            nc.scalar.dma_start(ident_sb[:], ident_ap[:])
            nc.scalar.dma_start(wo_sb[:], wo_ap[:])
            qkv_phase(0, xss0)
            xss1 = attn_phase(0, prefetch=1)
            qkv_phase(1, xss1)
            for mc in range(N_MCHUNK_B):
                outproj_chunk(0, mc)
            tc.no_sync_barrier()
            attn_phase(1)
            tc.no_sync_barrier()
            for mc in range(N_MCHUNK_B):
                outproj_chunk(1, mc)
    nc.compile()
    return nc


def _shard_inputs(x, w_qkv, b_qkv, w_out):
    import ml_dtypes

    bf = ml_dtypes.bfloat16
    xt = np.ascontiguousarray(x.reshape(M, D).T).astype(bf)  # (1024, 4096)
    ident = np.eye(128, dtype=bf)
    in_maps = []
    for c in range(N_CORES):
        h0 = HEADS_PER_CORE * c
        rows_q, rows_k, rows_v, dcols = [], [], [], []
        for h in (h0, h0 + 1):
            rows_q += list(range(h * 192, h * 192 + 64))
            rows_k += list(range(h * 192 + 64, h * 192 + 128))
            rows_v += list(range(h * 192 + 128, h * 192 + 192))
            dcols += list(range(h * 64, (h + 1) * 64))
        w_all = np.concatenate([
            np.ascontiguousarray(w_qkv[rows_q, :].T),
            np.ascontiguousarray(w_qkv[rows_k, :].T),
            np.ascontiguousarray(w_qkv[rows_v, :].T)], axis=0).astype(bf)
        b_all = np.stack([b_qkv[rows_q], b_qkv[rows_k], b_qkv[rows_v]],
                         axis=1).astype(np.float32)
        in_maps.append({
            "xt": xt,
            "w_all": np.ascontiguousarray(w_all),
            "wo": np.ascontiguousarray(w_out[:, dcols].T).astype(bf),
            "b_all": np.ascontiguousarray(b_all),
            "ident": ident,
        })
    return in_maps


def kernel(x, w_qkv, b_qkv, w_out, b_out, _trace=False):
    from concourse.bass_utils import run_bass_kernel_spmd

    x = np.asarray(x, dtype=np.float32)
    w_qkv = np.asarray(w_qkv, dtype=np.float32)
    b_qkv = np.asarray(b_qkv, dtype=np.float32)
    w_out = np.asarray(w_out, dtype=np.float32)
    b_out = np.asarray(b_out, dtype=np.float32)

    if "nc" not in _CACHE:
        _CACHE["nc"] = _build_module()
    nc = _CACHE["nc"]

    in_maps = _shard_inputs(x, w_qkv, b_qkv, w_out)
    res = run_bass_kernel_spmd(nc, in_maps, list(range(N_CORES)), trace=_trace)
    acc = np.zeros((D, M), dtype=np.float64)
    for c in range(N_CORES):
        acc += res.results[c]["partial"].astype(np.float32)
    acc = acc.T + b_out
    out = acc.astype(np.float32).reshape(B, S, D)
    if _trace:
        _CACHE["last_exec_time_ns"] = res.exec_time_ns
        _CACHE["last_res"] = res
    return out
